# revision 19
# baseline (speedup 1.0000x reference)
"""Trainium2 Bass kernel for nn_EpiNN_aaindex (pairwise-MLP GNN reduction).

Math (per batch b):
  x1 = emb@tw + tb
  X[i,d] = emb[i*64+d] * tw[i*64+d]            (L=256, D=64)
  s_ij = MLP(concat[(x_i+x_j)/2, |x_i-x_j|])   (128->64->16->1, LeakyReLU 0.01)
  out_b = x1 + scale * sum_{i<j} s_ij

Strategy: 8 cores, 4 batches/core (data parallel over B=32).
Exact upper-triangle enumeration via cyclic offsets o=1..128:
pairs (i, (i+o) mod 256) for o=1..127 cover each unordered pair once;
o=128 covers each of its 128 pairs twice (weighted 0.5 on the host).

Layouts (per batch, SBUF, bf16; X2T = X.T [64 d, 256 i]):
  XU  [128, 256]: both lane halves = X2T
  XSN [128, 512]: lanes 0:64 = X2T|X2T, lanes 64:128 = -(X2T|X2T)
Main loop p=0..7 pairs, 16 offsets/pair: o = 8p+1+b (+64j), j in {0,1},
b in 0..7 (sub-iter A: b 0..3, B: b 4..7).
  M [128, 16, 256] = XU - XSN[:, win] (ONE DVE op; windows via custom AP
    [[64,2],[1,8],[1,256]] starting at col 8p+1):
    block m=8j+b: lanes 0:64 = x_i - x_{i+o}, lanes 64:128 = x_i + x_{i+o}
  abs on lanes 0:64 (bitmask), then per half (A/B) 4 512-free
  128-contraction matmuls with stationary [w1b.T ; 0.5*w1a.T] ->
  P1 [128, 1024] (lanes 0:64 = j=0 offsets, 64:128 = j=1),
  act1 Lrelu+b1 -> H1 bf16 [128, 1024],
  L2: 4 matmuls, stationary = block-diag w2.T pairs at cols 32b:32b+32 ->
  P2D [128, 512] (A cols 0:256, B 256:512; lane r: b=r//32, j=(r%32)//16,
  f=r%16).
  One act2 Lrelu+b2 per pair with accum_out -> ACC[:, p].
o=128 (p=7, B, b=7, j=1): its L2 is split so only i<128 contributes
(each pair once); cols 384:512 of lanes 112:128 are zero-filled, adding
exactly 128*lrelu(b2) to ACC[112:128, 7] (host subtracts).

Final combine on host: out = x1 + tb + scale*(w3 . R + 32640*b3).
"""
import numpy as np

L, D = 256, 64
B_PER_CORE = 4
N_CORES = 8
NPAIRS = 32640  # 256*255/2

_CACHE = {}
import os as _os
N_BATCH = int(_os.environ.get("EPINN_BATCH", str(B_PER_CORE)))
N_RUN_CORES = int(_os.environ.get("EPINN_CORES", str(N_CORES)))
N_ITERS = int(_os.environ.get("EPINN_ITERS", "16"))


def _build_program():
    import concourse.bacc as bacc
    import concourse.mybir as mybir
    import concourse.tile as tile
    from concourse.bass import AP
    from contextlib import ExitStack

    f32 = mybir.dt.float32
    bf16 = mybir.dt.bfloat16
    u16 = mybir.dt.uint16
    AF = mybir.ActivationFunctionType
    ALU = mybir.AluOpType

    nc = bacc.Bacc("TRN2", target_bir_lowering=False, debug=False,
                   num_devices=N_CORES)

    # ---- DRAM parameters (per core) ----
    emb_d = nc.declare_dram_parameter("emb4", [B_PER_CORE, L * D + 1], f32,
                                      isOutput=False)
    w1fd_d = nc.declare_dram_parameter("w1fd", [128, 128], bf16, isOutput=False)
    w2d4_d = nc.declare_dram_parameter("w2d4", [128, 128], bf16, isOutput=False)
    w2t_d = nc.declare_dram_parameter("w2t2", [128, 64], bf16, isOutput=False)
    b1s_d = nc.declare_dram_parameter("b1s", [128, 1], f32, isOutput=False)
    b2s_d = nc.declare_dram_parameter("b2s", [128, 1], f32, isOutput=False)
    twp_d = nc.declare_dram_parameter("twp", [L, D], f32, isOutput=False)
    twl_d = nc.declare_dram_parameter("twl", [1, 1], f32, isOutput=False)

    acc_o = nc.declare_dram_parameter("acc_o", [B_PER_CORE, 128, 8], f32,
                                      isOutput=True)
    x1_o = nc.declare_dram_parameter("x1_o", [B_PER_CORE, 1, 1], f32,
                                     isOutput=True)

    with tile.TileContext(nc) as tc, ExitStack() as ctx:
        cpool = ctx.enter_context(tc.tile_pool(name="consts", bufs=1))
        ppool = ctx.enter_context(tc.tile_pool(name="persist", bufs=1))
        xpool = ctx.enter_context(tc.tile_pool(name="xbufs", bufs=2))
        mpool = ctx.enter_context(tc.tile_pool(name="mbufs", bufs=3))
        hpool = ctx.enter_context(tc.tile_pool(name="hbufs", bufs=3))
        jpool = ctx.enter_context(tc.tile_pool(name="junk", bufs=2))
        opool = ctx.enter_context(tc.tile_pool(name="outs", bufs=2))
        pp1 = ctx.enter_context(tc.tile_pool(name="p1", bufs=2, space="PSUM"))
        pp2 = ctx.enter_context(tc.tile_pool(name="p2", bufs=2, space="PSUM"))
        ppt = ctx.enter_context(tc.tile_pool(name="pt", bufs=2, space="PSUM"))

        # ---- static weights / consts ----
        W1FD = cpool.tile([128, 128], bf16)
        W2D4 = cpool.tile([128, 128], bf16)
        W2T = cpool.tile([128, 64], bf16)
        B1S = cpool.tile([128, 1], f32)
        B2S = cpool.tile([128, 1], f32)
        TWP = cpool.tile([128, 2, 64], f32)  # [128p, (half, d)]
        TWL = cpool.tile([1, 1], f32)
        IDENT = cpool.tile([128, 128], f32)
        ONES = cpool.tile([128, 1], f32)

        nc.sync.dma_start(W1FD[:], w1fd_d[:])
        nc.sync.dma_start(W2D4[:], w2d4_d[:])
        nc.sync.dma_start(W2T[:], w2t_d[:])
        nc.sync.dma_start(B1S[:], b1s_d[:])
        nc.sync.dma_start(B2S[:], b2s_d[:])
        # TWP halves: rows i=0..127 (half 0), i=128..255 (half 1)
        nc.sync.dma_start(TWP[:, 0, :], twp_d[0:128, :])
        nc.sync.dma_start(TWP[:, 1, :], twp_d[128:256, :])
        nc.sync.dma_start(TWL[:], twl_d[:])
        nc.gpsimd.memset(IDENT[:], 0.0)
        nc.gpsimd.affine_select(
            out=IDENT[:], in_=IDENT[:], compare_op=ALU.not_equal, fill=1.0,
            base=0, pattern=[[-1, 128]], channel_multiplier=1,
        )
        nc.gpsimd.memset(ONES[:], 1.0)

        XUs, XSNs, ACCs = [None] * N_BATCH, [None] * N_BATCH, [None] * N_BATCH

        def emit_setup(b):
            # ---- load emb row, build X = emb*tw [256i, 64d] as 2 tiles ----
            E2 = xpool.tile([128, 2, 64], f32, tag="e2")
            nc.sync.dma_start(
                E2[:], emb_d[b, 0:L * D].rearrange("(h p f) -> p h f", p=128, f=64)
            )
            EL = xpool.tile([1, 1], f32, tag="el")
            nc.sync.dma_start(EL[:], emb_d[b, L * D:L * D + 1][None, :])

            X2F = xpool.tile([128, 2, 64], f32, tag="x2f")
            nc.vector.tensor_tensor(out=X2F[:], in0=E2[:], in1=TWP[:],
                                    op=ALU.mult)

            # ---- x1 = sum(X2F) + EL*twl (tb added on host) ----
            CS = xpool.tile([128, 2], f32, tag="cs")
            nc.vector.tensor_reduce(out=CS[:, 0:1], in_=X2F[:, 0, :],
                                    op=ALU.add, axis=mybir.AxisListType.X)
            nc.vector.tensor_reduce(out=CS[:, 1:2], in_=X2F[:, 1, :],
                                    op=ALU.add, axis=mybir.AxisListType.X)
            CS1 = xpool.tile([128, 1], f32, tag="cs1")
            nc.vector.tensor_tensor(out=CS1[:], in0=CS[:, 0:1], in1=CS[:, 1:2],
                                    op=ALU.add)
            PTX = ppt.tile([64, 260], f32, tag="pt")
            PT = PTX[:, 0:256]
            PX1 = PTX[0:1, 256:257]
            nc.tensor.matmul(PX1[:], CS1[:], ONES[:], start=True, stop=False,
                             skip_group_check=True)
            nc.tensor.matmul(PX1[:], EL[:], TWL[:], start=False, stop=True,
                             skip_group_check=True)
            X1S = xpool.tile([1, 1], f32, tag="x1s")
            nc.scalar.copy(X1S[:], PX1[:])
            nc.sync.dma_start(x1_o[b], X1S[:])

            # ---- transpose X -> X2T [64d, 256i] in psum ----
            nc.tensor.matmul(PT[:, 0:128], X2F[:, 0, :], IDENT[:],
                             is_transpose=True, start=True, stop=True,
                             skip_group_check=True)
            nc.tensor.matmul(PT[:, 128:256], X2F[:, 1, :], IDENT[:],
                             is_transpose=True, start=True, stop=True,
                             skip_group_check=True)

            # ---- build XU [128,256] / XSN [128,512] (bf16) ----
            XU = ppool.tile([128, 256], bf16, name=f"xu{b}")
            XN = xpool.tile([64, 256], bf16, tag="xn")
            nc.scalar.copy(XU[0:64, :], PT[:])           # X2T
            nc.scalar.mul(XN[:], PT[:], -1.0)            # -X2T
            nc.sync.dma_start(XU[64:128, :], XU[0:64, :])
            XSN = ppool.tile([128, 512], bf16, name=f"xsn{b}")
            nc.sync.dma_start(XSN[0:64, 0:256], XU[0:64, :])
            nc.sync.dma_start(XSN[0:64, 256:512], XU[0:64, :])
            nc.sync.dma_start(XSN[64:128, 0:256], XN[:])
            nc.sync.dma_start(XSN[64:128, 256:512], XN[:])

            ACC = ppool.tile([128, 8], f32, name=f"acc{b}")
            nc.gpsimd.memset(ACC[:], 0.0)
            XUs[b] = XU
            XSNs[b] = XSN
            ACCs[b] = ACC

        def emit_pair(b, p):
            """One pair of sub-iterations: 16 offsets 8p+1..8p+8 (+64)."""
            XU, XSN, ACC = XUs[b], XSNs[b], ACCs[b]
            pstride = XU[:].ap.copy()[0][0]
            XU_B = AP(XU.tensor, XU.offset,
                      [[pstride, 128], [0, 2], [0, 8], [1, 256]])
            xsn_pstride = XSN[:].ap.copy()[0][0]

            o0 = 8 * p + 1
            M = mpool.tile([128, 16, 256], bf16, tag="m")
            MW = M[:].rearrange("p (a b) f -> p a b f", a=2)
            XSN_W = AP(XSN.tensor, XSN.offset + o0,
                       [[xsn_pstride, 128], [64, 2], [1, 8], [1, 256]])
            nc.vector.tensor_tensor(out=MW, in0=XU_B, in1=XSN_W,
                                    op=ALU.subtract)
            # abs on diff lanes (0:64)
            nc.vector.tensor_scalar(
                out=M[0:64, :, :].bitcast(u16),
                in0=M[0:64, :, :].bitcast(u16),
                scalar1=0x7FFF, scalar2=None, op0=ALU.bitwise_and)

            P2D = pp2.tile([128, 512], f32, tag="p2")
            P1s, H1s = [], []
            for h in (0, 1):  # sub-iteration A/B
                # ---- L1: 4 independent 128-contraction matmuls (512 free) ----
                P1 = pp1.tile([128, 1024], f32, tag="p1")
                for j in (0, 1):
                    lhs = W1FD[:, 64 * j:64 * j + 64]
                    for bb in (0, 2):
                        m0 = 8 * j + 4 * h + bb
                        nc.tensor.matmul(
                            P1[64 * j:64 * j + 64, 256 * bb:256 * bb + 512],
                            lhs, M[:, m0:m0 + 2, :],
                            start=True, stop=True, skip_group_check=True)
                P1s.append(P1)

            for h in (0, 1):
                H1 = hpool.tile([128, 1024], bf16, tag="h1")
                nc.scalar.activation(H1[:], P1s[h][:], AF.Lrelu, bias=B1S[:],
                                     scale=1.0, alpha=0.01)
                H1s.append(H1)

            for h in (0, 1):
                H1 = H1s[h]
                # ---- L2: 4 matmuls -> P2D cols 256h:256h+256 ----
                for bb in range(4):
                    if p == 7 and h == 1 and bb == 3:
                        # o=128 block: only i<128 (each pair once);
                        # lanes 112:128 cols 384:512 become zero-filled.
                        nc.tensor.matmul(
                            P2D[96:128, 256:512], W2T[:, 0:32],
                            H1[:, 768:1024],
                            start=True, stop=False, skip_group_check=True,
                            tile_position=(0, 96))
                        nc.tensor.matmul(
                            P2D[96:128, 256:384], W2T[:, 32:64],
                            H1[:, 768:896],
                            start=False, stop=True, skip_group_check=True,
                            tile_position=(0, 96))
                    else:
                        nc.tensor.matmul(
                            P2D[32 * bb:32 * bb + 32, 256 * h:256 * h + 256],
                            W2D4[:, 32 * bb:32 * bb + 32],
                            H1[:, 256 * bb:256 * bb + 256],
                            start=True, stop=True, skip_group_check=True,
                            tile_position=(0, 32 * bb))

            HJ = jpool.tile([128, 512], bf16, tag="hj")
            nc.scalar.activation(HJ[:], P2D[:], AF.Lrelu, bias=B2S[:],
                                 scale=1.0, alpha=0.01,
                                 accum_out=ACC[:, p:p + 1])

        emit_setup(0)
        for b in range(N_BATCH):
            for p in range(8):
                emit_pair(b, p)
                if p == 0 and b + 1 < N_BATCH:
                    emit_setup(b + 1)
            nc.sync.dma_start(acc_o[b], ACCs[b][:])

    nc.compile()
    return nc


def _get_program():
    key = (N_ITERS, N_BATCH)
    if key not in _CACHE:
        _CACHE[key] = _build_program()
    return _CACHE[key]


def _get_runner():
    """Build (once) a cached jitted SPMD executable for the program."""
    key = ("runner", N_ITERS, N_BATCH, N_RUN_CORES)
    if key in _CACHE:
        return _CACHE[key]
    import jax
    import numpy as _np
    import concourse.mybir as mybir
    from jax.sharding import Mesh, PartitionSpec
    from jax.experimental.shard_map import shard_map
    from concourse import bass2jax
    from concourse.bass2jax import _bass_exec_p, partition_id_tensor

    bass2jax.install_neuronx_cc_hook()
    nc = _get_program()
    n_cores = N_RUN_CORES

    partition_name = (nc.partition_id_tensor.name
                      if nc.partition_id_tensor else None)
    in_names, out_names, out_avals, zero_shapes = [], [], [], []
    for alloc in nc.m.functions[0].allocations:
        if not isinstance(alloc, mybir.MemoryLocationSet):
            continue
        name = alloc.memorylocations[0].name
        if alloc.kind == "ExternalInput":
            if name != partition_name:
                in_names.append(name)
        elif alloc.kind == "ExternalOutput":
            out_names.append(name)
            shape = tuple(alloc.tensor_shape)
            dtype = mybir.dt.np(alloc.dtype)
            out_avals.append(jax.core.ShapedArray(shape, dtype))
            zero_shapes.append((shape, dtype))
    n_params = len(in_names)
    n_outs = len(out_avals)
    all_in_names = list(in_names) + list(out_names)
    if partition_name is not None:
        all_in_names.append(partition_name)
    donate = tuple(range(n_params, n_params + n_outs))

    def _body(*args):
        operands = list(args)
        if partition_name is not None:
            operands.append(partition_id_tensor())
        outs = _bass_exec_p.bind(
            *operands, out_avals=tuple(out_avals), in_names=tuple(all_in_names),
            out_names=tuple(out_names), lowering_input_output_aliases=(),
            sim_require_finite=True, sim_require_nnan=True, nc=nc)
        return tuple(outs)

    devices = jax.devices()[:n_cores]
    mesh = Mesh(_np.asarray(devices), ("core",))
    in_specs = (PartitionSpec("core"),) * (n_params + n_outs)
    out_specs = (PartitionSpec("core"),) * len(out_names)
    sharded = jax.jit(
        shard_map(_body, mesh=mesh, in_specs=in_specs, out_specs=out_specs,
                  check_rep=False),
        donate_argnums=donate, keep_unused=True)

    def run(in_maps):
        concat_in = [
            np.concatenate([np.asarray(in_maps[c][nm]) for c in range(n_cores)],
                           axis=0)
            for nm in in_names
        ]
        concat_zeros = [np.zeros((n_cores * s[0], *s[1:]), d)
                        for (s, d) in zero_shapes]
        out_arrs = sharded(*concat_in, *concat_zeros)
        return [
            {nm: np.asarray(out_arrs[i]).reshape(n_cores, *out_avals[i].shape)[c]
             for i, nm in enumerate(out_names)}
            for c in range(n_cores)
        ]

    _CACHE[key] = run
    return run


def _prep_inputs(emb, tw, w1, b1, w2, b2):
    import ml_dtypes
    bfl = ml_dtypes.bfloat16

    w1 = np.asarray(w1, np.float32)
    w1f = np.concatenate([w1[:, 64:].T, 0.5 * w1[:, :64].T], axis=0)  # [128,64]
    w1fd = np.concatenate([w1f, w1f], axis=1).astype(bfl)             # [128,128]

    w2f = np.asarray(w2, np.float32)
    w2d4 = np.zeros((128, 128), np.float32)
    for bb in range(4):
        w2d4[0:64, 32 * bb:32 * bb + 16] = w2f.T
        w2d4[64:128, 32 * bb + 16:32 * bb + 32] = w2f.T
    w2d4 = w2d4.astype(bfl)
    w2t2 = np.zeros((128, 64), np.float32)
    w2t2[0:64, 0:16] = w2f.T          # W2J0: j=0 slot, j=1 zero
    w2t2[64:128, 48:64] = w2f.T       # W2J1: j=1 slot, j=0 zero
    w2t2 = w2t2.astype(bfl)

    b1v = np.asarray(b1, np.float32)
    b2v = np.asarray(b2, np.float32)
    b1s = np.concatenate([b1v, b1v]).reshape(128, 1).astype(np.float32)
    b2s = np.tile(b2v, 8).reshape(128, 1).astype(np.float32)
    twp = np.ascontiguousarray(tw[:-1].reshape(L, D)).astype(np.float32)
    twl = np.array([[tw[-1]]], np.float32)
    return {
        "w1fd": w1fd, "w2d4": w2d4, "w2t2": w2t2, "b1s": b1s, "b2s": b2s,
        "twp": twp, "twl": twl,
    }


def kernel(emb, tw, tb, w1, b1, w2, b2, w3, b3, scale):
    run = _get_runner()

    emb = np.asarray(emb, np.float32)
    tw = np.asarray(tw, np.float32)

    shared = _prep_inputs(emb, tw, w1, b1, w2, b2)
    in_maps = []
    for c in range(N_CORES):
        m = dict(shared)
        m["emb4"] = np.ascontiguousarray(emb[c * B_PER_CORE:(c + 1) * B_PER_CORE])
        in_maps.append(m)

    core_results = run(in_maps[:N_RUN_CORES])

    w3v = np.asarray(w3, np.float32)[0]
    b2v = np.asarray(b2, np.float32)
    # zero-filled block contributes 128*lrelu(b2) to lanes 112:128 col 7
    zero_corr = 128.0 * np.where(b2v > 0, b2v, 0.01 * b2v)
    out = np.zeros(32, np.float32)
    for c in range(N_RUN_CORES):
        r = core_results[c]
        acc = r["acc_o"]            # [4, 128, 8]
        x1p = r["x1_o"][:, 0, 0]    # [4]
        for b in range(N_BATCH):
            m16 = acc[b]                       # [128, 8]
            # lane r = (slot r//16, feature r%16); sum slots+cols
            R = m16.reshape(8, 16, 8).sum(axis=(0, 2)) - zero_corr
            out[c * B_PER_CORE + b] = (
                x1p[b] + float(tb[0])
                + float(scale[0]) * (R @ w3v + float(b3[0]) * NPAIRS)
            )
    return out
